# revision 3
# baseline (speedup 1.0000x reference)
"""BitLinear kernel for Trainium2 (8 NeuronCores, tensor-parallel).

Computes: out = x @ (sign(w) * mean(|w|, axis=1, keepdims=True)).T
  x      : [4, 2048, 4096] f32
  weight : [4096, 4096] f32
  out    : [4, 2048, 4096] f32

Strategy: shard weight rows (out features) 8-way; each core computes a
[512, 8192] feature-major output shard.

Mixed-precision contraction (PE issue rate is the bottleneck: every
matmul instruction — bf16/fp16 1-ktile or fp8 DoubleRow 2-ktile —
issues at ~216ns for 512 moving elements, so time = instruction count):
the first 14 k-tiles run as fp16 matmuls, the last 18 k-tiles run as 9
fp8e4 DoubleRow pair-matmuls. sign(w) is exact in fp16 and fp8e4, so
quantization error comes from the e4m3 x tiles; 18/32 fp8 k-tiles
lands just under the 2e-2 rel-err gate. Host computes signs and f32
scales; the device applies the per-feature scale while evicting PSUM
and stores the shard in fp16.

Per-pair x rides ONE big DMA per dtype (semaphore count on TRN2 is
~1 per DMA and both the NEFF preamble and the teardown semaphore-clear
loop scale with it); q0 is split finer so compute starts as the first
chunks land, with the critical first chunks on the gpsimd queue whose
preamble finishes ~1.5us before sync/scalar's.
"""

import os
from contextlib import ExitStack

import numpy as np
import ml_dtypes

import concourse.bass as bass
import concourse.mybir as mybir
import concourse.tile as tile
from concourse import bacc, bass_utils

P = 128                 # SBUF partitions / PE array dim
D_IN = 4096             # contraction dim (in features)
D_OUT = 4096            # out features
M_TOT = 8192            # tokens (4*2048)
N_CORES = 8
N_SHARD = D_OUT // N_CORES      # 512 out features per core
K_TILES = D_IN // P             # 32
NB = 14                         # fp16 k-tiles (0..NB-1)
NFP = (K_TILES - NB) // 2       # 9 fp8 DoubleRow k-tile pairs
M_BLK = 512                     # moving free dim per matmul
M_BLKS = M_TOT // M_BLK         # 16
M_PAIRS = M_BLKS // 2           # 8 (x is loaded in block pairs)
N_TILES = N_SHARD // P          # 4
PAIR_W = 2 * M_BLK              # 1024

_CACHE = {}
LAST_RESULTS = None  # BassKernelResults of the most recent run (for test harness)


def _install_ntff_hook():
    """Register the ctypes NTFF profiling hook under antenv.axon_hooks so
    run_bass_kernel_spmd(trace=True) can capture device profiles under axon.
    No-op if already present or the .so lacks the symbols."""
    import contextlib
    import ctypes
    import sys
    import types

    try:
        from antenv.axon_hooks import get_axon_ntff_profile_hook  # noqa: F401

        return True
    except ImportError:
        pass

    so_path = "/opt/axon/libaxon_pjrt.so"
    if not os.path.exists(so_path):
        return False
    lib = ctypes.CDLL(so_path)
    if not hasattr(lib, "axon_start_nrt_profile"):
        return False
    lib.axon_start_nrt_profile.argtypes = [
        ctypes.POINTER(ctypes.c_int64),
        ctypes.c_size_t,
    ]
    lib.axon_start_nrt_profile.restype = ctypes.c_int64
    lib.axon_stop_nrt_profile.argtypes = [ctypes.c_char_p]
    lib.axon_stop_nrt_profile.restype = ctypes.c_int64

    @contextlib.contextmanager
    def _hook(output_dir, device_ids):
        import jax

        jax.devices()
        if device_ids:
            ids = (ctypes.c_int64 * len(device_ids))(*device_ids)
            rc = lib.axon_start_nrt_profile(ids, len(device_ids))
        else:
            rc = lib.axon_start_nrt_profile(None, 0)
        if rc != 0:
            raise RuntimeError(f"axon_start_nrt_profile rc={rc}")
        try:
            yield
        finally:
            n = lib.axon_stop_nrt_profile(str(output_dir).encode())
            print(f"ntff profile: {n} file(s) written to {output_dir}")

    mod = types.ModuleType("antenv.axon_hooks")
    _state = {"hook": _hook}
    mod.set_axon_ntff_profile_hook = lambda h: _state.__setitem__("hook", h)
    mod.get_axon_ntff_profile_hook = lambda: _state["hook"]
    sys.modules["antenv.axon_hooks"] = mod
    import antenv

    antenv.axon_hooks = mod

    # artifact upload reaches for a cloud bucket that isn't available here
    bass_utils.upload_artifacts = lambda tmpdir: f"local:{tmpdir}"
    return True


def _build_nc():
    nc = bacc.Bacc(
        "TRN2", target_bir_lowering=False, debug=False, num_devices=N_CORES,
        enable_partition_id=False,
    )
    xH = nc.dram_tensor(
        "xH", [M_PAIRS, P, NB * PAIR_W], mybir.dt.float16, kind="ExternalInput"
    )
    xF = nc.dram_tensor(
        "xF", [M_PAIRS, P, NFP * 2 * PAIR_W], mybir.dt.float8e4,
        kind="ExternalInput",
    )
    sgB = nc.dram_tensor(
        "sgB", [P, NB * N_SHARD], mybir.dt.float16, kind="ExternalInput"
    )
    sgF = nc.dram_tensor(
        "sgF", [P, NFP * 2 * N_SHARD], mybir.dt.float8e4, kind="ExternalInput"
    )
    sc = nc.dram_tensor("sc", [P, N_TILES], mybir.dt.float32, kind="ExternalInput")
    outT = nc.dram_tensor(
        "outT", [P, N_TILES, M_TOT], mybir.dt.float16, kind="ExternalOutput"
    )

    with tile.TileContext(nc) as tc, ExitStack() as ctx:
        sb = ctx.enter_context(tc.tile_pool(name="sb", bufs=1))
        pp = ctx.enter_context(tc.tile_pool(name="psum", bufs=1, space="PSUM"))

        # Every tile allocated exactly once; reuse is explicit by parity.
        sgB_t = sb.tile([P, NB, N_SHARD], mybir.dt.float16)
        sgF_t = sb.tile([P, NFP, 2, N_SHARD], mybir.dt.float8e4)
        sct = sb.tile([P, N_TILES], mybir.dt.float32)
        warm = sb.tile([P, P + M_BLK], mybir.dt.float16)
        xbs = [
            sb.tile([P, NB, PAIR_W], mybir.dt.float16, name=f"xb{i}")
            for i in range(2)
        ]
        xfs = [
            sb.tile([P, NFP, 2, PAIR_W], mybir.dt.float8e4, name=f"xf{i}")
            for i in range(2)
        ]
        ops = [
            sb.tile([P, N_TILES, PAIR_W], mybir.dt.float16, name=f"op{i}")
            for i in range(2)
        ]
        # 8 PSUM banks as two 4-bank sets; block g (= 2q+b) uses set g%2.
        psums = [
            [
                pp.tile([P, M_BLK], mybir.dt.float32, name=f"ps{i}_{ni}")
                for ni in range(N_TILES)
            ]
            for i in range(2)
        ]

        # Per-queue DMA emission-order chains.
        prev_dma = {}

        def qload(queue, qname, dst, src):
            dma = queue.dma_start(dst, src)
            if prev_dma.get(qname) is not None:
                tile.add_dep_helper(
                    dma.ins, prev_dma[qname].ins, sync=False,
                    reason="DMA queue emission order",
                )
            prev_dma[qname] = dma
            return dma

        # ---- HAM warmup: two dummy matmuls with no DMA dependencies give
        # the PE clock-gate monitor activity credit while the first x/sign
        # chunks land; sized to finish as the operands become consumable.
        nc.vector.memset(warm[:], 0.0)
        for wi in range(2):
            nc.tensor.matmul(
                psums[0][wi][:], warm[:, 0:P], warm[:, P : P + M_BLK],
                start=True, stop=True,
            )

        xb0, xf0 = xbs[0], xfs[0]
        # Critical first chunks on the gpsimd queue (earliest preamble end).
        qload(nc.gpsimd, "g", sgB_t[:, 0, :], sgB[:, 0:N_SHARD])
        qload(nc.gpsimd, "g", xb0[:, 0, 0:M_BLK], xH[0, :, 0:M_BLK])
        qload(nc.gpsimd, "g", xb0[:, 0, M_BLK:PAIR_W], xH[0, :, M_BLK:PAIR_W])
        qload(nc.gpsimd, "g", xb0[:, 1, :], xH[0, :, PAIR_W : 2 * PAIR_W])
        # Remaining signs + scales on the scalar queue.
        qload(nc.scalar, "s", sgB_t[:, 1:4, :], sgB[:, N_SHARD : 4 * N_SHARD])
        qload(nc.scalar, "s", sgB_t[:, 4:9, :], sgB[:, 4 * N_SHARD : 9 * N_SHARD])
        qload(nc.scalar, "s", sgB_t[:, 9:NB, :], sgB[:, 9 * N_SHARD : NB * N_SHARD])
        qload(nc.scalar, "s", sgF_t[:, 0:4, :, :], sgF[:, 0 : 8 * N_SHARD])
        qload(nc.scalar, "s", sgF_t[:, 4:NFP, :, :], sgF[:, 8 * N_SHARD :])
        qload(nc.scalar, "s", sct[:], sc[:, :])
        # Rest of pair 0's x on the sync queue, in consumption order.
        qload(nc.sync, "y", xb0[:, 2:4, :], xH[0, :, 2 * PAIR_W : 4 * PAIR_W])
        qload(nc.sync, "y", xb0[:, 4:7, :], xH[0, :, 4 * PAIR_W : 7 * PAIR_W])
        qload(nc.sync, "y", xb0[:, 7:10, :], xH[0, :, 7 * PAIR_W : 10 * PAIR_W])
        qload(nc.sync, "y", xb0[:, 10:NB, :], xH[0, :, 10 * PAIR_W :])
        qload(nc.sync, "y", xf0[:, 0:4, :, :], xF[0, :, 0 : 4 * 2 * PAIR_W])
        qload(nc.sync, "y", xf0[:, 4:NFP, :, :], xF[0, :, 4 * 2 * PAIR_W :])

        def issue_x_pair(q):
            xb, xf = xbs[q % 2], xfs[q % 2]
            qload(nc.sync, "y", xb[:, :, :], xH[q, :, :])
            qload(nc.sync, "y", xf[:, :, :, :], xF[q, :, :])
            return xb, xf

        def mm_b(pss, xb, b, ni, j):
            nc.tensor.matmul(
                pss[ni][:],
                sgB_t[:, j, ni * P : (ni + 1) * P],
                xb[:, j, b * M_BLK : (b + 1) * M_BLK],
                start=(j == 0),
                stop=False,
            )

        def mm_f(pss, xf, b, ni, jj):
            nc.tensor.matmul(
                pss[ni][:],
                sgF_t[:, jj, :, ni * P : (ni + 1) * P],
                xf[:, jj, :, b * M_BLK : (b + 1) * M_BLK],
                start=False,
                stop=(jj == NFP - 1),
                perf_mode=mybir.MatmulPerfMode.DoubleRow,
            )

        def evict_block(pss, op, b):
            # Evictions alternate between the scalar and vector engines so
            # the per-block eviction chain (and the kernel tail) is half as
            # long.
            for ni in range(N_TILES):
                dst = op[:, ni, b * M_BLK : (b + 1) * M_BLK]
                if ni % 2 == 0:
                    nc.scalar.mul(dst, pss[ni][:], sct[:, ni : ni + 1])
                else:
                    nc.vector.tensor_scalar_mul(dst, pss[ni][:], sct[:, ni : ni + 1])

        def store_pair(q, op):
            qload(
                nc.scalar, "s",
                outT[:, :, q * PAIR_W : (q + 1) * PAIR_W], op[:, :, :],
            )

        # ---- Main loop
        for q in range(M_PAIRS):
            xb, xf = (xb0, xf0) if q == 0 else issue_x_pair(q)
            op = ops[q % 2]
            if q < M_PAIRS - 1:
                for b in range(2):
                    pss = psums[b]
                    for j in range(NB):
                        for ni in range(N_TILES):
                            mm_b(pss, xb, b, ni, j)
                    for jj in range(NFP):
                        for ni in range(N_TILES):
                            mm_f(pss, xf, b, ni, jj)
                    evict_block(pss, op, b)
                store_pair(q, op)
            else:
                # Final pair: block 0 stores as one chunk as soon as its
                # eviction completes; block 1 runs ni-outer so each n-tile's
                # stop matmul lands early and its eviction + store overlap
                # the remaining matmuls; the very last n-tile's eviction and
                # store are split across both engines / two DMA queues.
                pss = psums[0]
                for j in range(NB):
                    for ni in range(N_TILES):
                        mm_b(pss, xb, 0, ni, j)
                for jj in range(NFP):
                    for ni in range(N_TILES):
                        mm_f(pss, xf, 0, ni, jj)
                evict_block(pss, op, 0)
                qload(
                    nc.scalar, "s",
                    outT[:, :, q * PAIR_W : q * PAIR_W + M_BLK],
                    op[:, :, 0:M_BLK],
                )
                pss = psums[1]
                tailq = [(nc.gpsimd, "g"), (nc.sync, "y"), (nc.scalar, "s")]
                for ni in range(N_TILES):
                    for j in range(NB):
                        mm_b(pss, xb, 1, ni, j)
                    for jj in range(NFP):
                        mm_f(pss, xf, 1, ni, jj)
                    c0 = q * PAIR_W + M_BLK
                    if ni < N_TILES - 1:
                        dst = op[:, ni, M_BLK:PAIR_W]
                        if ni % 2 == 0:
                            nc.scalar.mul(dst, pss[ni][:], sct[:, ni : ni + 1])
                        else:
                            nc.vector.tensor_scalar_mul(
                                dst, pss[ni][:], sct[:, ni : ni + 1]
                            )
                        eng, en = tailq[ni]
                        qload(
                            eng, en,
                            outT[:, ni, c0 : c0 + M_BLK],
                            op[:, ni, M_BLK:PAIR_W],
                        )
                    else:
                        # split the last eviction + store across engines
                        H = M_BLK // 2
                        nc.scalar.mul(
                            op[:, ni, M_BLK : M_BLK + H],
                            pss[ni][:, 0:H], sct[:, ni : ni + 1],
                        )
                        nc.vector.tensor_scalar_mul(
                            op[:, ni, M_BLK + H : PAIR_W],
                            pss[ni][:, H:M_BLK], sct[:, ni : ni + 1],
                        )
                        qload(
                            nc.gpsimd, "g",
                            outT[:, ni, c0 : c0 + H],
                            op[:, ni, M_BLK : M_BLK + H],
                        )
                        qload(
                            nc.sync, "y",
                            outT[:, ni, c0 + H : c0 + M_BLK],
                            op[:, ni, M_BLK + H : PAIR_W],
                        )

    nc.compile()
    return nc


def kernel(x, weight):
    global LAST_RESULTS
    nc = _CACHE.get("nc")
    if nc is None:
        nc = _CACHE["nc"] = _build_nc()

    x = np.asarray(x)
    weight = np.asarray(weight)
    orig_shape = x.shape

    KB = NB * P  # contraction cols in fp16

    # Host-side layout: x.T pre-tiled, partition-major so each pair is one
    # contiguous [128, *] DMA; fp16 for k-tiles 0..NB-1, e4m3 for the
    # DoubleRow k-tile pairs.
    xT = x.reshape(M_TOT, D_IN).T  # [D_IN, M_TOT] view
    # [q, p, j*1024 + c] = xT[j*128+p, q*1024+c]
    xH = np.ascontiguousarray(
        xT[:KB].reshape(NB, P, M_PAIRS, PAIR_W)
        .transpose(2, 1, 0, 3)
        .reshape(M_PAIRS, P, NB * PAIR_W)
        .astype(np.float16)
    )
    # [q, p, jj*2048 + t*1024 + c] = xT[KB + (2jj+t)*128 + p, q*1024+c]
    xF = np.ascontiguousarray(
        xT[KB:].reshape(NFP, 2, P, M_PAIRS, PAIR_W)
        .transpose(3, 2, 0, 1, 4)
        .reshape(M_PAIRS, P, NFP * 2 * PAIR_W)
        .astype(ml_dtypes.float8_e4m3fn)
    )

    SgT = np.sign(weight.T)  # [D_IN, D_OUT] f32, sign exact
    s_full = np.abs(weight.astype(np.float64)).mean(axis=1).astype(np.float32)

    in_maps = []
    for c in range(N_CORES):
        n0 = c * N_SHARD
        shard = SgT[:, n0 : n0 + N_SHARD]  # [D_IN, 512]
        # sgB[p, j*512+n] = sign(wT[j*128+p, n0+n])
        sgB = np.ascontiguousarray(
            shard[:KB].reshape(NB, P, N_SHARD)
            .transpose(1, 0, 2)
            .reshape(P, NB * N_SHARD)
            .astype(np.float16)
        )
        # sgF[p, jj*1024 + t*512 + n] = sign(wT[(NB+2jj+t)*128+p, n0+n])
        sgF = np.ascontiguousarray(
            shard[KB:].reshape(NFP, 2, P, N_SHARD)
            .transpose(2, 0, 1, 3)
            .reshape(P, NFP * 2 * N_SHARD)
            .astype(ml_dtypes.float8_e4m3fn)
        )
        scl = np.ascontiguousarray(
            s_full[n0 : n0 + N_SHARD].reshape(N_TILES, P).T
        )  # [128, 4] f32
        in_maps.append({"xH": xH, "xF": xF, "sgB": sgB, "sgF": sgF, "sc": scl})

    trace = bool(int(os.environ.get("BITLIN_TRACE", "0")))
    if trace:
        trace = _install_ntff_hook()
        base = os.environ.get("BITLIN_TRACE_DIR") or None
        if base:
            import tempfile

            os.makedirs(base, exist_ok=True)
            tmpdir = tempfile.mkdtemp(dir=base)
        else:
            tmpdir = None
    else:
        tmpdir = None
    res = bass_utils.run_bass_kernel_spmd(
        nc, in_maps, core_ids=list(range(N_CORES)), trace=trace, tmpdir=tmpdir
    )
    LAST_RESULTS = res

    # outT[c] is [128, 4, 8192] fp16 with feature index = ni*128 + p.
    outT_full = np.concatenate(
        [
            np.asarray(res.results[c]["outT"]).transpose(1, 0, 2).reshape(
                N_SHARD, M_TOT
            )
            for c in range(N_CORES)
        ],
        axis=0,
    )  # [D_OUT, M_TOT] fp16
    out = (
        np.ascontiguousarray(outT_full.T).astype(np.float32).reshape(orig_shape)
    )
    return out


# revision 9
# speedup vs baseline: 1.0081x; 1.0081x over previous
"""BitLinear kernel for Trainium2 (8 NeuronCores, tensor-parallel).

Computes: out = x @ (sign(w) * mean(|w|, axis=1, keepdims=True)).T
  x      : [4, 2048, 4096] f32
  weight : [4096, 4096] f32
  out    : [4, 2048, 4096] f32

Strategy: shard weight rows (out features) 8-way; each core computes a
[512, 8192] feature-major output shard.

Mixed-precision contraction (PE issue rate is the bottleneck: every
matmul instruction — bf16/fp16 1-ktile or fp8 DoubleRow 2-ktile —
issues at ~216ns for 512 moving elements, so time = instruction count):
the first 14 k-tiles run as fp16 matmuls, the last 18 k-tiles run as 9
fp8e4 DoubleRow pair-matmuls. sign(w) is exact in fp16 and fp8e4, so
quantization error comes from the e4m3 x tiles; 18/32 fp8 k-tiles
lands just under the 2e-2 rel-err gate. Host computes signs and f32
scales; the device applies the per-feature scale while evicting PSUM
and stores the shard in fp16.

Per-pair x rides ONE big DMA per dtype (semaphore count on TRN2 is
~1 per DMA and both the NEFF preamble and the teardown semaphore-clear
loop scale with it); q0 is split finer so compute starts as the first
chunks land, with the critical first chunks on the gpsimd queue whose
preamble finishes ~1.5us before sync/scalar's.
"""

import os
from contextlib import ExitStack

import numpy as np
import ml_dtypes

import concourse.bass as bass
import concourse.mybir as mybir
import concourse.tile as tile
from concourse import bacc, bass_utils

P = 128                 # SBUF partitions / PE array dim
D_IN = 4096             # contraction dim (in features)
D_OUT = 4096            # out features
M_TOT = 8192            # tokens (4*2048)
N_CORES = 8
N_SHARD = D_OUT // N_CORES      # 512 out features per core
K_TILES = D_IN // P             # 32
NB = 14                         # fp16 k-tiles (0..NB-1)
NFP = (K_TILES - NB) // 2       # 9 fp8 DoubleRow k-tile pairs
M_BLK = 512                     # moving free dim per matmul
M_BLKS = M_TOT // M_BLK         # 16
M_PAIRS = M_BLKS // 2           # 8 (x is loaded in block pairs)
N_TILES = N_SHARD // P          # 4
PAIR_W = 2 * M_BLK              # 1024

_CACHE = {}
LAST_RESULTS = None  # BassKernelResults of the most recent run (for test harness)


def _install_ntff_hook():
    """Register the ctypes NTFF profiling hook under antenv.axon_hooks so
    run_bass_kernel_spmd(trace=True) can capture device profiles under axon.
    No-op if already present or the .so lacks the symbols."""
    import contextlib
    import ctypes
    import sys
    import types

    try:
        from antenv.axon_hooks import get_axon_ntff_profile_hook  # noqa: F401

        return True
    except ImportError:
        pass

    so_path = "/opt/axon/libaxon_pjrt.so"
    if not os.path.exists(so_path):
        return False
    lib = ctypes.CDLL(so_path)
    if not hasattr(lib, "axon_start_nrt_profile"):
        return False
    lib.axon_start_nrt_profile.argtypes = [
        ctypes.POINTER(ctypes.c_int64),
        ctypes.c_size_t,
    ]
    lib.axon_start_nrt_profile.restype = ctypes.c_int64
    lib.axon_stop_nrt_profile.argtypes = [ctypes.c_char_p]
    lib.axon_stop_nrt_profile.restype = ctypes.c_int64

    @contextlib.contextmanager
    def _hook(output_dir, device_ids):
        import jax

        jax.devices()
        if device_ids:
            ids = (ctypes.c_int64 * len(device_ids))(*device_ids)
            rc = lib.axon_start_nrt_profile(ids, len(device_ids))
        else:
            rc = lib.axon_start_nrt_profile(None, 0)
        if rc != 0:
            raise RuntimeError(f"axon_start_nrt_profile rc={rc}")
        try:
            yield
        finally:
            n = lib.axon_stop_nrt_profile(str(output_dir).encode())
            print(f"ntff profile: {n} file(s) written to {output_dir}")

    mod = types.ModuleType("antenv.axon_hooks")
    _state = {"hook": _hook}
    mod.set_axon_ntff_profile_hook = lambda h: _state.__setitem__("hook", h)
    mod.get_axon_ntff_profile_hook = lambda: _state["hook"]
    sys.modules["antenv.axon_hooks"] = mod
    import antenv

    antenv.axon_hooks = mod

    # artifact upload reaches for a cloud bucket that isn't available here
    bass_utils.upload_artifacts = lambda tmpdir: f"local:{tmpdir}"
    return True


def _build_nc():
    nc = bacc.Bacc(
        "TRN2", target_bir_lowering=False, debug=False, num_devices=N_CORES,
        enable_partition_id=False,
    )
    xH = nc.dram_tensor(
        "xH", [M_PAIRS, P, NB, PAIR_W], mybir.dt.float16, kind="ExternalInput"
    )
    xF = nc.dram_tensor(
        "xF", [M_PAIRS, P, NFP, 2, PAIR_W], mybir.dt.float8e4,
        kind="ExternalInput",
    )
    sgB = nc.dram_tensor(
        "sgB", [P, NB * N_SHARD], mybir.dt.float16, kind="ExternalInput"
    )
    sgF = nc.dram_tensor(
        "sgF", [P, NFP * 2 * N_SHARD], mybir.dt.float8e4, kind="ExternalInput"
    )
    sc = nc.dram_tensor("sc", [P, N_TILES], mybir.dt.float32, kind="ExternalInput")
    outT = nc.dram_tensor(
        "outT", [P, N_TILES, M_TOT], mybir.dt.float16, kind="ExternalOutput"
    )

    with tile.TileContext(nc) as tc, ExitStack() as ctx:
        sb = ctx.enter_context(tc.tile_pool(name="sb", bufs=1))
        pp = ctx.enter_context(tc.tile_pool(name="psum", bufs=1, space="PSUM"))

        # Every tile allocated exactly once; reuse is explicit by parity.
        sgB_t = sb.tile([P, NB, N_SHARD], mybir.dt.float16)
        sgF_t = sb.tile([P, NFP, 2, N_SHARD], mybir.dt.float8e4)
        sct = sb.tile([P, N_TILES], mybir.dt.float32)
        warm = sb.tile([P, P + M_BLK], mybir.dt.float16)
        xbs = [
            sb.tile([P, NB, PAIR_W], mybir.dt.float16, name=f"xb{i}")
            for i in range(2)
        ]
        xfs = [
            sb.tile([P, NFP, 2, PAIR_W], mybir.dt.float8e4, name=f"xf{i}")
            for i in range(2)
        ]
        ops = [
            sb.tile([P, N_TILES, PAIR_W], mybir.dt.float16, name=f"op{i}")
            for i in range(2)
        ]
        # 8 PSUM banks as two 4-bank sets; block g (= 2q+b) uses set g%2.
        psums = [
            [
                pp.tile([P, M_BLK], mybir.dt.float32, name=f"ps{i}_{ni}")
                for ni in range(N_TILES)
            ]
            for i in range(2)
        ]

        # Per-queue DMA emission-order chains.
        prev_dma = {}

        def qload(queue, qname, dst, src):
            dma = queue.dma_start(dst, src)
            if prev_dma.get(qname) is not None:
                tile.add_dep_helper(
                    dma.ins, prev_dma[qname].ins, sync=False,
                    reason="DMA queue emission order",
                )
            prev_dma[qname] = dma
            return dma

        # ---- HAM warmup: three dummy matmuls with no DMA dependencies give
        # the PE clock-gate monitor activity credit while the first x/sign
        # chunks land; sized to finish as the operands become consumable.
        nc.vector.memset(warm[:], 0.0)
        for wi in range(3):
            nc.tensor.matmul(
                psums[0][wi][:], warm[:, 0:P], warm[:, P : P + M_BLK],
                start=True, stop=True,
            )

        xb0, xf0 = xbs[0], xfs[0]
        # Pair 0's x on the sync queue in exact block-0 consumption order:
        # h0 halves of every fp16 k-tile, then h0 of the fp8 pairs, then the
        # h1 halves as two strided bulk DMAs (block 1 runs ~20us later).
        # (gpsimd DMAs are software-executed with multi-us completion
        # latency — never put them on the critical path.)
        qload(nc.sync, "y", xb0[:, 0, 0:M_BLK], xH[0, :, 0, 0:M_BLK])
        qload(nc.sync, "y", xb0[:, 1, 0:M_BLK], xH[0, :, 1, 0:M_BLK])
        for j0 in range(2, NB, 3):
            j1 = min(j0 + 3, NB)
            qload(
                nc.sync, "y", xb0[:, j0:j1, 0:M_BLK], xH[0, :, j0:j1, 0:M_BLK]
            )
        for c0 in range(0, NFP, 3):
            c1 = min(c0 + 3, NFP)
            qload(
                nc.sync, "y", xf0[:, c0:c1, :, 0:M_BLK],
                xF[0, :, c0:c1, :, 0:M_BLK],
            )
        qload(nc.sync, "y", xb0[:, :, M_BLK:PAIR_W], xH[0, :, :, M_BLK:PAIR_W])
        qload(
            nc.sync, "y", xf0[:, :, :, M_BLK:PAIR_W], xF[0, :, :, :, M_BLK:PAIR_W]
        )
        # Signs + scales on the scalar queue, first k-tile first.
        qload(nc.scalar, "s", sgB_t[:, 0, :], sgB[:, 0:N_SHARD])
        qload(nc.scalar, "s", sgB_t[:, 1:4, :], sgB[:, N_SHARD : 4 * N_SHARD])
        qload(nc.scalar, "s", sgB_t[:, 4:9, :], sgB[:, 4 * N_SHARD : 9 * N_SHARD])
        qload(nc.scalar, "s", sgB_t[:, 9:NB, :], sgB[:, 9 * N_SHARD : NB * N_SHARD])
        qload(nc.scalar, "s", sgF_t[:, 0:4, :, :], sgF[:, 0 : 8 * N_SHARD])
        qload(nc.scalar, "s", sgF_t[:, 4:NFP, :, :], sgF[:, 8 * N_SHARD :])
        qload(nc.scalar, "s", sct[:], sc[:, :])

        def issue_x_pair(q):
            xb, xf = xbs[q % 2], xfs[q % 2]
            qload(nc.sync, "y", xb[:, :, :], xH[q, :, :, :])
            qload(nc.sync, "y", xf[:, :, :, :], xF[q, :, :, :, :])
            return xb, xf

        def mm_b(pss, xb, b, ni, j):
            nc.tensor.matmul(
                pss[ni][:],
                sgB_t[:, j, ni * P : (ni + 1) * P],
                xb[:, j, b * M_BLK : (b + 1) * M_BLK],
                start=(j == 0),
                stop=False,
            )

        def mm_f(pss, xf, b, ni, jj):
            nc.tensor.matmul(
                pss[ni][:],
                sgF_t[:, jj, :, ni * P : (ni + 1) * P],
                xf[:, jj, :, b * M_BLK : (b + 1) * M_BLK],
                start=False,
                stop=(jj == NFP - 1),
                perf_mode=mybir.MatmulPerfMode.DoubleRow,
            )

        def evict_block(pss, op, b):
            # Evictions alternate between the scalar and vector engines so
            # the per-block eviction chain (and the kernel tail) is half as
            # long.
            for ni in range(N_TILES):
                dst = op[:, ni, b * M_BLK : (b + 1) * M_BLK]
                if ni % 2 == 0:
                    nc.scalar.mul(dst, pss[ni][:], sct[:, ni : ni + 1])
                else:
                    nc.vector.tensor_scalar_mul(dst, pss[ni][:], sct[:, ni : ni + 1])

        def store_pair(q, op):
            qload(
                nc.scalar, "s",
                outT[:, :, q * PAIR_W : (q + 1) * PAIR_W], op[:, :, :],
            )

        # ---- Main loop
        for q in range(M_PAIRS):
            xb, xf = (xb0, xf0) if q == 0 else issue_x_pair(q)
            op = ops[q % 2]
            if q < M_PAIRS - 1:
                for b in range(2):
                    pss = psums[b]
                    for j in range(NB):
                        for ni in range(N_TILES):
                            mm_b(pss, xb, b, ni, j)
                    for jj in range(NFP):
                        for ni in range(N_TILES):
                            mm_f(pss, xf, b, ni, jj)
                    evict_block(pss, op, b)
                store_pair(q, op)
            else:
                # Final pair: block 0 stores as one chunk as soon as its
                # eviction completes; block 1 runs ni-outer so each n-tile's
                # stop matmul lands early and its eviction + store overlap
                # the remaining matmuls; the very last n-tile's eviction and
                # store are split across both engines / two DMA queues.
                pss = psums[0]
                for j in range(NB):
                    for ni in range(N_TILES):
                        mm_b(pss, xb, 0, ni, j)
                for jj in range(NFP):
                    for ni in range(N_TILES):
                        mm_f(pss, xf, 0, ni, jj)
                evict_block(pss, op, 0)
                qload(
                    nc.scalar, "s",
                    outT[:, :, q * PAIR_W : q * PAIR_W + M_BLK],
                    op[:, :, 0:M_BLK],
                )
                pss = psums[1]
                tailq = [(nc.scalar, "s"), (nc.sync, "y"), (nc.scalar, "s")]
                for ni in range(N_TILES):
                    for j in range(NB):
                        mm_b(pss, xb, 1, ni, j)
                    for jj in range(NFP):
                        mm_f(pss, xf, 1, ni, jj)
                    c0 = q * PAIR_W + M_BLK
                    if ni < N_TILES - 1:
                        dst = op[:, ni, M_BLK:PAIR_W]
                        if ni % 2 == 0:
                            nc.scalar.mul(dst, pss[ni][:], sct[:, ni : ni + 1])
                        else:
                            nc.vector.tensor_scalar_mul(
                                dst, pss[ni][:], sct[:, ni : ni + 1]
                            )
                        eng, en = tailq[ni]
                        qload(
                            eng, en,
                            outT[:, ni, c0 : c0 + M_BLK],
                            op[:, ni, M_BLK:PAIR_W],
                        )
                    else:
                        # split the last eviction + store across engines
                        H = M_BLK // 2
                        nc.scalar.mul(
                            op[:, ni, M_BLK : M_BLK + H],
                            pss[ni][:, 0:H], sct[:, ni : ni + 1],
                        )
                        nc.vector.tensor_scalar_mul(
                            op[:, ni, M_BLK + H : PAIR_W],
                            pss[ni][:, H:M_BLK], sct[:, ni : ni + 1],
                        )
                        qload(
                            nc.scalar, "s",
                            outT[:, ni, c0 : c0 + H],
                            op[:, ni, M_BLK : M_BLK + H],
                        )
                        qload(
                            nc.sync, "y",
                            outT[:, ni, c0 + H : c0 + M_BLK],
                            op[:, ni, M_BLK + H : PAIR_W],
                        )

    nc.compile()
    return nc


def kernel(x, weight):
    global LAST_RESULTS
    nc = _CACHE.get("nc")
    if nc is None:
        nc = _CACHE["nc"] = _build_nc()

    x = np.asarray(x)
    weight = np.asarray(weight)
    orig_shape = x.shape

    KB = NB * P  # contraction cols in fp16

    # Host-side layout: x.T pre-tiled, partition-major so each pair is one
    # contiguous [128, *] DMA; fp16 for k-tiles 0..NB-1, e4m3 for the
    # DoubleRow k-tile pairs.
    xT = x.reshape(M_TOT, D_IN).T  # [D_IN, M_TOT] view
    # [q, p, j*1024 + c] = xT[j*128+p, q*1024+c]
    xH = np.ascontiguousarray(
        xT[:KB].reshape(NB, P, M_PAIRS, PAIR_W)
        .transpose(2, 1, 0, 3)
        .reshape(M_PAIRS, P, NB * PAIR_W)
        .astype(np.float16)
    )
    # [q, p, jj*2048 + t*1024 + c] = xT[KB + (2jj+t)*128 + p, q*1024+c]
    xF = np.ascontiguousarray(
        xT[KB:].reshape(NFP, 2, P, M_PAIRS, PAIR_W)
        .transpose(3, 2, 0, 1, 4)
        .reshape(M_PAIRS, P, NFP * 2 * PAIR_W)
        .astype(ml_dtypes.float8_e4m3fn)
    )

    SgT = np.sign(weight.T)  # [D_IN, D_OUT] f32, sign exact
    s_full = np.abs(weight.astype(np.float64)).mean(axis=1).astype(np.float32)

    in_maps = []
    for c in range(N_CORES):
        n0 = c * N_SHARD
        shard = SgT[:, n0 : n0 + N_SHARD]  # [D_IN, 512]
        # sgB[p, j*512+n] = sign(wT[j*128+p, n0+n])
        sgB = np.ascontiguousarray(
            shard[:KB].reshape(NB, P, N_SHARD)
            .transpose(1, 0, 2)
            .reshape(P, NB * N_SHARD)
            .astype(np.float16)
        )
        # sgF[p, jj*1024 + t*512 + n] = sign(wT[(NB+2jj+t)*128+p, n0+n])
        sgF = np.ascontiguousarray(
            shard[KB:].reshape(NFP, 2, P, N_SHARD)
            .transpose(2, 0, 1, 3)
            .reshape(P, NFP * 2 * N_SHARD)
            .astype(ml_dtypes.float8_e4m3fn)
        )
        scl = np.ascontiguousarray(
            s_full[n0 : n0 + N_SHARD].reshape(N_TILES, P).T
        )  # [128, 4] f32
        in_maps.append({"xH": xH, "xF": xF, "sgB": sgB, "sgF": sgF, "sc": scl})

    trace = bool(int(os.environ.get("BITLIN_TRACE", "0")))
    if trace:
        trace = _install_ntff_hook()
        base = os.environ.get("BITLIN_TRACE_DIR") or None
        if base:
            import tempfile

            os.makedirs(base, exist_ok=True)
            tmpdir = tempfile.mkdtemp(dir=base)
        else:
            tmpdir = None
    else:
        tmpdir = None
    res = bass_utils.run_bass_kernel_spmd(
        nc, in_maps, core_ids=list(range(N_CORES)), trace=trace, tmpdir=tmpdir
    )
    LAST_RESULTS = res

    # outT[c] is [128, 4, 8192] fp16 with feature index = ni*128 + p.
    outT_full = np.concatenate(
        [
            np.asarray(res.results[c]["outT"]).transpose(1, 0, 2).reshape(
                N_SHARD, M_TOT
            )
            for c in range(N_CORES)
        ],
        axis=0,
    )  # [D_OUT, M_TOT] fp16
    out = (
        np.ascontiguousarray(outT_full.T).astype(np.float32).reshape(orig_shape)
    )
    return out
